# revision 1
# baseline (speedup 1.0000x reference)
"""GAT (2-layer, 8-head) Trainium2 Bass kernel, SPMD over 8 NeuronCores.

Sharding: node rows of the attention matrix are sharded 384/core
(N=3000 padded to 3072 = 24 j-tiles of 128). Each core computes
h = X@W for all nodes (replicated bf16 matmuls), then softmax rows for
its shard against all nodes. Scores are built transposed, E^T[j, i],
so the att@h contraction (over j) is the PE partition dim -- no big
transposes. exp(lrelu(z)) = max(exp(z), exp(alpha z)): two wide ACT Exp
passes per j-tile, max+mask on DVE (bf16 2x). Denominator comes from a
ones-column in the stationary operand of the same matmul, with a
per-row diagonal fixup (a1 = A+I is 2 where adj has a diagonal).
Two small AllGathers: node degrees, and layer-2 (h2|src2|dst2).
Phase order is chosen so the src/dst scalars (sd) are ready early --
the ACT engine (the long pole, ~170us of exp) starts ~25us in.
"""

import numpy as np

N = 3000
NP = 3072            # padded node count = 24 * 128
S = 384              # rows per core   = 3 * 128
NCORES = 8
IN_DIM = 512
HID = 64
HEADS = 8
NCLS = 16
JT = NP // 128       # 24 j-tiles
RT = S // 128        # 3 row-tiles
ALPHA = 0.2
EPS = 1e-6

_CACHE = {}


def _build_nc(loop_n=None):
    import concourse.bass as bass
    import concourse.bacc as bacc
    import concourse.mybir as mybir
    from concourse import tile

    dt = mybir.dt
    f32 = dt.float32
    bf16 = dt.bfloat16
    AF = mybir.ActivationFunctionType
    OP = mybir.AluOpType
    AX = mybir.AxisListType

    nc = bacc.Bacc("TRN2", target_bir_lowering=False, debug=False,
                   num_devices=NCORES)

    # ---------------- DRAM I/O ----------------
    adjc = nc.dram_tensor("adjc", [NP, S], f32, kind="ExternalInput")
    xT = nc.dram_tensor("xT", [IN_DIM, NP], f32, kind="ExternalInput")
    xTown = nc.dram_tensor("xTown", [IN_DIM, S], f32, kind="ExternalInput")
    diagv = nc.dram_tensor("diagv", [S, 1], f32, kind="ExternalInput")
    w_all = nc.dram_tensor("w_all", [IN_DIM, 512], f32, kind="ExternalInput")
    whT = nc.dram_tensor("whT", [512, IN_DIM], f32, kind="ExternalInput")
    a2h = nc.dram_tensor("a2h", [HID, 16], f32, kind="ExternalInput")
    w_out = nc.dram_tensor("w_out", [512, NCLS], f32, kind="ExternalInput")
    woT = nc.dram_tensor("woT", [NCLS, 512], f32, kind="ExternalInput")
    a2o = nc.dram_tensor("a2o", [NCLS, 2], f32, kind="ExternalInput")
    fc1T = nc.dram_tensor("fc1T", [NCLS, NCLS], f32, kind="ExternalInput")
    fc2T = nc.dram_tensor("fc2T", [NCLS, NCLS], f32, kind="ExternalInput")
    ident = nc.dram_tensor("ident", [128, 128], f32, kind="ExternalInput")
    identb = nc.dram_tensor("identb", [128, 128], bf16, kind="ExternalInput")
    out_own = nc.dram_tensor("out_own", [S, NCLS], f32, kind="ExternalOutput")

    V = nc.vector
    SC = nc.scalar
    G = nc.gpsimd
    T = nc.tensor
    SY = nc.sync

    with tile.TileContext(nc) as tc:
        with tc.tile_pool(name="persist", bufs=1) as P, \
             tc.tile_pool(name="dram", bufs=1, space="DRAM") as D:

            # ---- persistent SBUF ----
            a1 = P.tile([128, JT * S], bf16, name="a1")          # a1^T (A+I), bf16
            a2h_sb = P.tile([HID, 16], bf16, name="a2h_sb")
            wo_all = P.tile([128, 4 * NCLS], bf16, name="wo_all")
            wo_bf = [wo_all[:, NCLS * k:NCLS * (k + 1)] for k in range(4)]
            a2o_sb = P.tile([NCLS, 2], bf16, name="a2o_sb")
            fc1_sb = P.tile([NCLS, NCLS], f32, name="fc1_sb")
            fc2_sb = P.tile([NCLS, NCLS], f32, name="fc2_sb")
            id_sb = P.tile([128, 128], f32, name="id_sb")
            idb_sb = P.tile([128, 128], bf16, name="idb_sb")
            dv_sb = P.tile([128, RT], f32, name="dv_sb")         # adjacency diag (own)
            ones_bf = P.tile([128, 1], bf16, name="ones_bf")
            epsv = P.tile([128, 1], f32, name="epsv")
            sdext = P.tile([128, JT * 16], f32, name="sdext")    # src/dst all nodes
            hd = P.tile([128, JT * 520], bf16, name="hd")        # dinv*h | 1 per head
            srcB8 = P.tile([128, HEADS * S], f32, name="srcB8")
            dinvj = P.tile([128, JT], f32, name="dinvj")
            dinvo = P.tile([128, RT], f32, name="dinvo")
            degow = P.tile([128, RT], f32, name="degow")
            degj = P.tile([128, JT], f32, name="degj")
            eq1 = P.tile([128, RT * HEADS], f32, name="eq1")
            xnat = [P.tile([128, 512], f32, name=f"xn{r}") for r in range(RT)]
            xt2 = [P.tile([128, S], bf16, name=f"xt2{k}") for k in range(4)]
            va2_bf = [P.tile([128, 2], bf16, name=f"va2{k}") for k in range(4)]
            gsb = P.tile([128, JT * 18], f32, name="gsb")
            hd2 = P.tile([128, JT * 17], bf16, name="hd2")
            srcB2 = P.tile([128, S], f32, name="srcB2")
            gown_sb = [P.tile([128, 18], f32, name=f"go{r}") for r in range(RT)]

            # ---- DRAM bounce tensors ----
            srcdram = D.tile([HEADS, S], f32, name="srcdram")
            src2dram = D.tile([1, S], f32, name="src2dram")
            degown_d = D.tile([S, 1], f32, name="degown_d")
            degfull_d = D.tile([NP, 1], f32, name="degfull_d")
            gown_d = D.tile([S, 18], f32, name="gown_d")
            gfull_d = D.tile([NP, 18], f32, name="gfull_d")

            def _phases():
                # ---- input DMAs, ordered for earliest consumers ----
                V.memset(ones_bf[:], 1.0)
                V.memset(epsv[:], EPS)
                G.dma_start(out=a2h_sb[:], in_=a2h[:])
                G.dma_start(out=a2o_sb[:], in_=a2o[:])
                SY.dma_start(out=id_sb[:], in_=ident[:])
                SY.dma_start(out=idb_sb[:], in_=identb[:])
                SY.dma_start(out=dv_sb[:].rearrange("p (r one) -> p r one", r=RT),
                             in_=diagv[:].rearrange("(r p) one -> p r one", p=128))
                SY.dma_start(out=fc1_sb[:], in_=fc1T[:])
                SY.dma_start(out=fc2_sb[:], in_=fc2T[:])

                with tc.tile_pool(name="wpool", bufs=1) as WP:
                    xtow_all = WP.tile([128, 4 * S], bf16, name="xtow_all")
                    wsb_all = WP.tile([128, 4 * 512], bf16, name="wsb_all")
                    whT_all = WP.tile([64, HEADS * 512], bf16, name="whT_all")
                    woT_sb = WP.tile([NCLS, 512], bf16, name="woT_sb")
                    xt_all = WP.tile([128, 4 * NP], bf16, name="xt_all")
                    own = WP.tile([128, RT * 528], f32, name="own")
                    va_sb = [WP.tile([128, 16], bf16, name=f"va{k}") for k in range(4)]
                    xtow_sb = [xtow_all[:, S * k:S * (k + 1)] for k in range(4)]
                    w_sb = [wsb_all[:, 512 * k:512 * (k + 1)] for k in range(4)]
                    whT_sb = [whT_all[:, 512 * h:512 * (h + 1)] for h in range(HEADS)]
                    xt_sb = [xt_all[:, NP * k:NP * (k + 1)] for k in range(4)]
                    G.dma_start(out=xtow_all[:].rearrange("p (k c) -> p k c", k=4),
                                in_=xTown[:].rearrange("(k p) c -> p k c", p=128))
                    G.dma_start(out=wsb_all[:].rearrange("p (k c) -> p k c", k=4),
                                in_=w_all[:].rearrange("(k p) c -> p k c", p=128))
                    G.dma_start(out=whT_all[:].rearrange("p (h c) -> p h c", h=HEADS),
                                in_=whT[:].rearrange("(h p) c -> p h c", p=64))
                    NQ = NP // 4
                    G.dma_start(
                        out=xt_all[:].rearrange("p (k c) -> p k c", k=4)[:, :, 0:NQ],
                        in_=xT[:].rearrange("(k p) c -> p k c", p=128)[:, :, 0:NQ])
                    G.dma_start(out=a1[:].rearrange("p (t s) -> p t s", t=JT),
                                in_=adjc[:].rearrange("(t p) s -> p t s", p=128))
                    for q in range(1, 4):
                        G.dma_start(
                            out=xt_all[:].rearrange("p (k c) -> p k c", k=4)
                            [:, :, NQ * q:NQ * (q + 1)],
                            in_=xT[:].rearrange("(k p) c -> p k c", p=128)
                            [:, :, NQ * q:NQ * (q + 1)])
                    G.dma_start(out=woT_sb[:], in_=woT[:])
                    G.dma_start(out=wo_all[:].rearrange("p (k c) -> p k c", k=4),
                                in_=w_out[:].rearrange("(k p) c -> p k c", p=128))

                    # ---- va = W_h @ [a1h a2h]; va2 = W_out @ [a1o a2o] ----
                    with tc.tile_pool(name="ps_va", bufs=2, space="PSUM") as PSV:
                        for k in range(4):
                            vps = PSV.tile([128, 16], f32, name="vps", tag="vps")
                            for h in range(HEADS):
                                T.matmul(vps[:, 2 * h:2 * h + 2],
                                         whT_sb[h][:, 128 * k:128 * (k + 1)],
                                         a2h_sb[:, 2 * h:2 * h + 2],
                                         start=True, stop=True)
                            V.tensor_copy(va_sb[k][:], vps[:])
                        for k in range(4):
                            vps2 = PSV.tile([128, 2], f32, name="vps2", tag="vps2")
                            T.matmul(vps2[:], woT_sb[:, 128 * k:128 * (k + 1)],
                                     a2o_sb[:], start=True, stop=True)
                            V.tensor_copy(va2_bf[k][:], vps2[:])

                    with tc.tile_pool(name="ps_h", bufs=2, space="PSUM") as PSH:
                        # ---- own rows first: h|sd -> srcB8 + eq1 early ----
                        for r in range(RT):
                            hxa = PSH.tile([128, 512], f32, name="hxa", tag="hxa")
                            hxb = PSH.tile([128, 16], f32, name="hxb", tag="hxb")
                            for k in range(4):
                                lhs = xtow_sb[k][:, 128 * r:128 * (r + 1)]
                                T.matmul(hxa[:], lhs, w_sb[k][:],
                                         start=(k == 0), stop=(k == 3))
                                T.matmul(hxb[:], lhs, va_sb[k][:],
                                         start=(k == 0), stop=(k == 3))
                            SC.copy(own[:, 528 * r:528 * r + 512], hxa[:])
                            V.tensor_copy(own[:, 528 * r + 512:528 * (r + 1)], hxb[:])

                        # srcB8: extract src columns, bounce via DRAM, broadcast
                        with tc.tile_pool(name="ps_s", bufs=2, space="PSUM") as PSS:
                            srcsT = WP.tile([HEADS, RT * 128], f32, name="srcsT")
                            for r in range(RT):
                                sps = PSS.tile([HEADS, 128], f32, name="sps",
                                               tag="sps")
                                T.matmul(sps[:],
                                         own[:, 528 * r + 512:528 * (r + 1):2],
                                         id_sb[:], start=True, stop=True,
                                         is_transpose=True)
                                V.tensor_copy(srcsT[:, 128 * r:128 * (r + 1)], sps[:])
                        SY.dma_start(out=srcdram[:], in_=srcsT[:])
                        SY.dma_start(
                            out=srcB8[:].rearrange("p (h s) -> p h s", h=HEADS),
                            in_=srcdram[:].unsqueeze(0).to_broadcast((128, HEADS, S)))
                        # eq1 = diag * exp(lrelu(src_i + dst_i))
                        zii = P.tile([128, RT * HEADS], f32, name="zii")
                        for r in range(RT):
                            V.tensor_tensor(zii[:, 8 * r:8 * (r + 1)],
                                            own[:, 528 * r + 512:528 * (r + 1):2],
                                            own[:, 528 * r + 513:528 * (r + 1):2],
                                            OP.add)
                        t1 = P.tile([128, RT * HEADS], f32, name="t1")
                        V.tensor_scalar(t1[:], zii[:], ALPHA, None, OP.mult)
                        V.tensor_tensor(t1[:], zii[:], t1[:], OP.max)
                        SC.activation(t1[:], t1[:], AF.Exp)
                        for r in range(RT):
                            V.tensor_scalar(eq1[:, 8 * r:8 * (r + 1)],
                                            t1[:, 8 * r:8 * (r + 1)],
                                            dv_sb[:, r:r + 1], None, OP.mult)

                        # ---- degrees -> AllGather -> dinv ----
                        with tc.tile_pool(name="ps_d", bufs=1, space="PSUM") as PSD:
                            dps = PSD.tile([128, RT], f32, name="dps")
                            for r in range(RT):
                                for t in range(JT):
                                    T.matmul(dps[:, r:r + 1],
                                             a1[:, S * t + 128 * r:
                                                S * t + 128 * (r + 1)],
                                             ones_bf[:], start=(t == 0),
                                             stop=(t == JT - 1))
                            V.tensor_copy(degow[:], dps[:])
                        SY.dma_start(
                            out=degown_d[:].rearrange("(r p) one -> p r one", p=128),
                            in_=degow[:].rearrange("p (r one) -> p r one", r=RT))
                        G.collective_compute(
                            "AllGather", OP.bypass,
                            replica_groups=[list(range(NCORES))],
                            ins=[degown_d[:].opt()], outs=[degfull_d[:].opt()])

                        # ---- sd + h interleaved per tile: each DVE copy
                        # waits only its own tile's matmuls (no in-order-DVE
                        # head-of-line on the copies) ----
                        for t in range(JT):
                            hxb = PSH.tile([128, 16], f32, name="hxb", tag="hxb")
                            for k in range(4):
                                T.matmul(hxb[:], xt_sb[k][:, 128 * t:128 * (t + 1)],
                                         va_sb[k][:], start=(k == 0), stop=(k == 3))
                            V.tensor_copy(sdext[:, 16 * t:16 * (t + 1)], hxb[:])
                            hxa = PSH.tile([128, 512], f32, name="hxa", tag="hxa")
                            for k in range(4):
                                T.matmul(hxa[:], xt_sb[k][:, 128 * t:128 * (t + 1)],
                                         w_sb[k][:], start=(k == 0), stop=(k == 3))
                            V.tensor_copy(
                                hd[:, 520 * t:520 * (t + 1)]
                                .rearrange("p (h c) -> p h c", h=HEADS)[:, :, 0:64],
                                hxa[:].rearrange("p (h c) -> p h c", h=HEADS))

                def _hd_scale():
                    SY.dma_start(
                        out=degj[:].rearrange("p (t one) -> p t one", t=JT),
                        in_=degfull_d[:].rearrange("(t p) one -> p t one", p=128))
                    # dinv = exp(-0.5 ln(deg+eps)) -- exp/ln table set; traced
                    # after the lead exps so the in-order ACT queue is not
                    # head-of-line blocked on the collective.
                    lnj = P.tile([128, JT], f32, name="lnj")
                    lno = P.tile([128, RT], f32, name="lno")
                    SC.activation(lnj[:], degj[:], AF.Ln, bias=epsv[:])
                    SC.activation(lno[:], degow[:], AF.Ln, bias=epsv[:])
                    SC.activation(dinvj[:], lnj[:], AF.Exp, scale=-0.5)
                    SC.activation(dinvo[:], lno[:], AF.Exp, scale=-0.5)
                    V.memset(hd[:].rearrange("p (t h c) -> p t h c", t=JT, h=HEADS)
                             [:, :, :, 64:65], 1.0)
                    # in-place dinv_j scale of the h part of hd (bf16)
                    for t in range(JT):
                        hslice = hd[:, 520 * t:520 * (t + 1)] \
                            .rearrange("p (h c) -> p h c", h=HEADS)[:, :, 0:64]
                        V.tensor_scalar(hslice, hslice, dinvj[:, t:t + 1],
                                        None, OP.mult)

                # ---- layer-1 attention ----
                with tc.tile_pool(name="zw", bufs=2) as ZW:
                  with tc.tile_pool(name="ps_att", bufs=1, space="PSUM") as PSA, \
                       tc.tile_pool(name="abw", bufs=2) as ABW:
                    ps1 = [PSA.tile([65, S], f32, name=f"ps1_{h}", tag=f"ps1_{h}")
                           for h in range(HEADS)]
                    o1s = [ZW.tile([65, S], f32, name=f"o1s{h}", tag=f"o1s{h}",
                                   bufs=1) for h in range(HEADS)]
                    def _elem(t):
                        z = ZW.tile([128, HEADS * S], f32, name="z", tag="z", bufs=3)
                        for h in range(HEADS):
                            V.tensor_scalar(z[:, S * h:S * (h + 1)],
                                            srcB8[:, S * h:S * (h + 1)],
                                            sdext[:, 16 * t + 2 * h + 1:
                                                  16 * t + 2 * h + 2],
                                            None, OP.add)
                        A = ABW.tile([128, HEADS * S], bf16, name="A", tag="A", bufs=3)
                        B = ABW.tile([128, HEADS * S], bf16, name="B", tag="B", bufs=3)
                        SC.activation(A[:], z[:], AF.Exp)
                        SC.activation(B[:], z[:], AF.Exp, scale=ALPHA)
                        Q = ABW.tile([128, HEADS * S], bf16, name="Q", tag="Q",
                                     bufs=3)
                        V.tensor_tensor(Q[:], A[:], B[:], OP.max)
                        R = ABW.tile([128, HEADS * S], bf16, name="R", tag="R",
                                     bufs=3)
                        EM = G if (t % 3 == 2) else V
                        EM.tensor_tensor(
                            R[:].rearrange("p (h s) -> p h s", h=HEADS),
                            Q[:].rearrange("p (h s) -> p h s", h=HEADS),
                            a1[:, S * t:S * (t + 1)].unsqueeze(1)
                            .to_broadcast((128, HEADS, S)),
                            OP.mult)
                        return R

                    def _mms(t, R):
                        for h in range(HEADS):
                            T.matmul(ps1[h][:],
                                     hd[:, 520 * t + 65 * h:520 * t + 65 * (h + 1)],
                                     R[:, S * h:S * (h + 1)],
                                     start=(t == 0), stop=(t == JT - 1))

                    LEAD = 3
                    lead_R = [_elem(t) for t in range(LEAD)]
                    _hd_scale()
                    for t in range(LEAD):
                        _mms(t, lead_R[t])
                    for t in range(LEAD, JT):
                        R = _elem(t)
                        _mms(t, R)
                    for h in range(HEADS):
                        V.tensor_copy(o1s[h][:], ps1[h][:])

                  # ---- layer-1 epilogue: scale + elu -> xnat, xt2 ----
                  with tc.tile_pool(name="ps_tr", bufs=4, space="PSUM") as PST:
                    for h in range(HEADS):
                        pt = PST.tile([128, RT * 65], f32, name="pt", tag="pt")
                        for r in range(RT):
                            T.matmul(pt[:, 65 * r:65 * (r + 1)],
                                     o1s[h][:, 128 * r:128 * (r + 1)],
                                     id_sb[0:65, 0:65],
                                     start=True, stop=True, is_transpose=True)
                        den = P.tile([128, RT], f32, name="den", tag="den", bufs=2)
                        V.scalar_tensor_tensor(
                            den[:], pt[:, 64:65 * RT:65], EPS,
                            eq1[:, h:8 * RT:8], OP.add, OP.subtract)
                        rec = P.tile([128, RT], f32, name="rec", tag="rec", bufs=2)
                        V.reciprocal(rec[:], den[:])
                        sc = P.tile([128, RT], f32, name="scl", tag="scl", bufs=2)
                        V.tensor_tensor(sc[:], rec[:], dinvo[:], OP.mult)
                        for r in range(RT):
                            SC.activation(xnat[r][:, 64 * h:64 * (h + 1)],
                                          pt[:, 65 * r:65 * r + 64],
                                          AF.Copy, scale=sc[:, r:r + 1])
                    for r in range(RT):
                        tmin = P.tile([128, 512], f32, name="tmin", tag="tmin", bufs=2)
                        V.tensor_scalar(tmin[:], xnat[r][:], 0.0, None, OP.min)
                        SC.activation(tmin[:], tmin[:], AF.Exp)
                        rl = P.tile([128, 512], f32, name="rl", tag="rl", bufs=2)
                        V.tensor_scalar(rl[:], xnat[r][:], 0.0, -1.0, OP.max, OP.add)
                        xb = P.tile([128, 512], bf16, name="xb", tag="xb", bufs=2)
                        V.tensor_tensor(xb[:], tmin[:], rl[:], OP.add)
                        for k in range(4):
                            ptx = PST.tile([128, 128], bf16, name="ptx", tag="ptx")
                            T.matmul(ptx[:], xb[:, 128 * k:128 * (k + 1)], idb_sb[:],
                                     start=True, stop=True, is_transpose=True)
                            V.tensor_copy(xt2[k][:, 128 * r:128 * (r + 1)], ptx[:])

                # ---- layer 2: h2|sd2 own rows -> AllGather ----
                with tc.tile_pool(name="ps_h2", bufs=2, space="PSUM") as PSH2:
                    for r in range(RT):
                        h2p = PSH2.tile([128, 16], f32, name="h2p", tag="h2p")
                        sd2p = PSH2.tile([128, 2], f32, name="sd2p", tag="sd2p")
                        for k in range(4):
                            lhs = xt2[k][:, 128 * r:128 * (r + 1)]
                            T.matmul(h2p[:], lhs, wo_bf[k][:],
                                     start=(k == 0), stop=(k == 3))
                            T.matmul(sd2p[:], lhs, va2_bf[k][:],
                                     start=(k == 0), stop=(k == 3))
                        V.tensor_copy(gown_sb[r][:, 0:16], h2p[:])
                        V.tensor_copy(gown_sb[r][:, 16:18], sd2p[:])
                        SY.dma_start(out=gown_d[128 * r:128 * (r + 1), :],
                                     in_=gown_sb[r][:])
                G.collective_compute("AllGather", OP.bypass,
                                     replica_groups=[list(range(NCORES))],
                                     ins=[gown_d[:].opt()], outs=[gfull_d[:].opt()])
                SY.dma_start(out=gsb[:].rearrange("p (t c) -> p t c", t=JT),
                             in_=gfull_d[:].rearrange("(t p) c -> p t c", p=128))

                # hd2 ones-columns (builds happen per-group inside the loop)
                V.memset(hd2[:].rearrange("p (t c) -> p t c", t=JT)[:, :, 16:17], 1.0)

                # srcB2 broadcast
                with tc.tile_pool(name="ps_s2", bufs=2, space="PSUM") as PSS2:
                    srcs2 = P.tile([1, RT * 128], f32, name="srcs2")
                    for r in range(RT):
                        sps2 = PSS2.tile([1, 128], f32, name="sps2", tag="sps2")
                        T.matmul(sps2[:], gown_sb[r][:, 16:17], id_sb[:],
                                 start=True, stop=True, is_transpose=True)
                        V.tensor_copy(srcs2[:, 128 * r:128 * (r + 1)], sps2[:])
                SY.dma_start(out=src2dram[:], in_=srcs2[:])
                SY.dma_start(out=srcB2[:], in_=src2dram[:].to_broadcast((128, S)))

                # eq2 = diag * exp(lrelu(src2 + dst2))
                eq2 = P.tile([128, RT], f32, name="eq2")
                z2i = P.tile([128, RT], f32, name="z2i")
                for r in range(RT):
                    V.tensor_tensor(z2i[:, r:r + 1], gown_sb[r][:, 16:17],
                                    gown_sb[r][:, 17:18], OP.add)
                t2i = P.tile([128, RT], f32, name="t2i")
                V.tensor_scalar(t2i[:], z2i[:], ALPHA, None, OP.mult)
                V.tensor_tensor(t2i[:], z2i[:], t2i[:], OP.max)
                SC.activation(t2i[:], t2i[:], AF.Exp)
                for r in range(RT):
                    V.tensor_scalar(eq2[:, r:r + 1], t2i[:, r:r + 1],
                                    dv_sb[:, r:r + 1], None, OP.mult)

                # ---- layer-2 attention (4 j-tiles per ACT call) ----
                with tc.tile_pool(name="ps_a2", bufs=1, space="PSUM") as PSA2, \
                     tc.tile_pool(name="zw2", bufs=2) as ZW2, \
                     tc.tile_pool(name="ab2", bufs=2) as AB2:
                    ps2 = PSA2.tile([17, S], f32, name="ps2")
                    GRP = 4
                    for g in range(JT // GRP):
                        z2 = ZW2.tile([128, GRP * S], f32, name="z2", tag="z2")
                        for i in range(GRP):
                            t = GRP * g + i
                            V.tensor_scalar(z2[:, S * i:S * (i + 1)], srcB2[:],
                                            gsb[:, 18 * t + 17:18 * t + 18],
                                            None, OP.add)
                        for i in range(GRP):
                            t = GRP * g + i
                            V.tensor_scalar(hd2[:, 17 * t:17 * t + 16],
                                            gsb[:, 18 * t:18 * t + 16],
                                            dinvj[:, t:t + 1], None, OP.mult)
                        A2 = AB2.tile([128, GRP * S], bf16, name="A2", tag="A2")
                        B2 = AB2.tile([128, GRP * S], bf16, name="B2", tag="B2")
                        SC.activation(A2[:], z2[:], AF.Exp)
                        SC.activation(B2[:], z2[:], AF.Exp, scale=ALPHA)
                        Q2 = AB2.tile([128, GRP * S], bf16, name="Q2", tag="Q2")
                        V.tensor_tensor(Q2[:], A2[:], B2[:], OP.max)
                        R2 = AB2.tile([128, GRP * S], bf16, name="R2", tag="R2")
                        V.tensor_tensor(R2[:], Q2[:],
                                        a1[:, S * GRP * g:S * GRP * (g + 1)],
                                        OP.mult)
                        for i in range(GRP):
                            t = GRP * g + i
                            T.matmul(ps2[:], hd2[:, 17 * t:17 * (t + 1)],
                                     R2[:, S * i:S * (i + 1)],
                                     start=(t == 0), stop=(t == JT - 1))
                    o2s = P.tile([17, S], f32, name="o2s")
                    V.tensor_copy(o2s[:], ps2[:])

                # ---- layer-2 epilogue + FC + log_softmax ----
                with tc.tile_pool(name="ps_e2", bufs=2, space="PSUM") as PSE:
                    pt2 = PSE.tile([128, RT * 17], f32, name="pt2")
                    for r in range(RT):
                        T.matmul(pt2[:, 17 * r:17 * (r + 1)],
                                 o2s[:, 128 * r:128 * (r + 1)], id_sb[0:17, 0:17],
                                 start=True, stop=True, is_transpose=True)
                    for r in range(RT):
                        den2 = P.tile([128, 1], f32, name="den2", tag="den2", bufs=2)
                        V.scalar_tensor_tensor(den2[:],
                                               pt2[:, 17 * r + 16:17 * r + 17],
                                               EPS, eq2[:, r:r + 1],
                                               OP.add, OP.subtract)
                        rec2 = P.tile([128, 1], f32, name="rec2", tag="rec2", bufs=2)
                        V.reciprocal(rec2[:], den2[:])
                        sc2 = P.tile([128, 1], f32, name="sc2", tag="sc2", bufs=2)
                        V.tensor_tensor(sc2[:], rec2[:], dinvo[:, r:r + 1], OP.mult)
                        hp2 = P.tile([128, NCLS], f32, name="hp2", tag="hp2", bufs=2)
                        SC.activation(hp2[:], pt2[:, 17 * r:17 * r + 16],
                                      AF.Copy, scale=sc2[:])
                        x2 = hp2
                        for _ in range(2):
                            tm = P.tile([128, NCLS], f32, name="tm2", tag="tm2",
                                        bufs=2)
                            V.tensor_scalar(tm[:], x2[:], 0.0, None, OP.min)
                            SC.activation(tm[:], tm[:], AF.Exp)
                            rl2 = P.tile([128, NCLS], f32, name="rl2", tag="rl2",
                                         bufs=2)
                            V.tensor_scalar(rl2[:], x2[:], 0.0, -1.0, OP.max, OP.add)
                            x2 = P.tile([128, NCLS], f32, name="x2e", tag="x2e",
                                        bufs=2)
                            V.tensor_tensor(x2[:], tm[:], rl2[:], OP.add)
                        for fc in (fc1_sb, fc2_sb):
                            xtp = PSE.tile([16, 128], f32, name="xtp", tag="xtp")
                            T.matmul(xtp[:], x2[:], id_sb[:], start=True, stop=True,
                                     is_transpose=True)
                            xts = P.tile([16, 128], f32, name="xts", tag="xts",
                                         bufs=2)
                            V.tensor_copy(xts[:], xtp[:])
                            fps = PSE.tile([128, NCLS], f32, name="fps", tag="fps")
                            T.matmul(fps[:], xts[:], fc[:], start=True, stop=True)
                            tm = P.tile([128, NCLS], f32, name="tm2", tag="tm2",
                                        bufs=2)
                            V.tensor_scalar(tm[:], fps[:], 0.0, None, OP.min)
                            SC.activation(tm[:], tm[:], AF.Exp)
                            rl2 = P.tile([128, NCLS], f32, name="rl2", tag="rl2",
                                         bufs=2)
                            V.tensor_scalar(rl2[:], fps[:], 0.0, -1.0, OP.max, OP.add)
                            x2 = P.tile([128, NCLS], f32, name="x2e", tag="x2e",
                                        bufs=2)
                            V.tensor_tensor(x2[:], tm[:], rl2[:], OP.add)
                        rmax = P.tile([128, 1], f32, name="rmax", tag="rmax", bufs=2)
                        V.tensor_reduce(rmax[:], x2[:], AX.X, OP.max)
                        u = P.tile([128, NCLS], f32, name="u", tag="u", bufs=2)
                        V.tensor_scalar(u[:], x2[:], rmax[:], None, OP.subtract)
                        eu = P.tile([128, NCLS], f32, name="eu", tag="eu", bufs=2)
                        ssum = P.tile([128, 1], f32, name="ssum", tag="ssum", bufs=2)
                        SC.activation(eu[:], u[:], AF.Exp, accum_out=ssum[:])
                        lg = P.tile([128, 1], f32, name="lg", tag="lg", bufs=2)
                        SC.activation(lg[:], ssum[:], AF.Ln)
                        outr = P.tile([128, NCLS], f32, name="outr", tag="outr",
                                      bufs=2)
                        V.tensor_scalar(outr[:], u[:], lg[:], None, OP.subtract)
                        SY.dma_start(out=out_own[128 * r:128 * (r + 1), :],
                                     in_=outr[:])

            if loop_n is None:
                _phases()
            else:
                with tc.For_i(0, loop_n, 1):
                    _phases()

    nc.compile()
    nc.finalize()
    return nc


def _prep_inputs(inputs):
    adjacency = np.asarray(inputs["adjacency"], np.float32)
    features = np.asarray(inputs["features"], np.float32)
    W_heads = np.asarray(inputs["W_heads"], np.float32)
    a_heads = np.asarray(inputs["a_heads"], np.float32)
    W_out = np.asarray(inputs["W_out"], np.float32)
    a_out = np.asarray(inputs["a_out"], np.float32)
    FC1 = np.asarray(inputs["FC1"], np.float32)
    FC2 = np.asarray(inputs["FC2"], np.float32)

    try:
        from ml_dtypes import bfloat16 as bf
    except ImportError:  # jax ships ml_dtypes
        import jax.numpy as jnp
        bf = jnp.bfloat16

    a1 = adjacency.copy()
    a1[np.arange(N), np.arange(N)] += 1.0          # A + I
    a1p = np.zeros((NP, NP), np.float32)
    a1p[:N, :N] = a1
    xTp = np.zeros((IN_DIM, NP), np.float32)
    xTp[:, :N] = features.T
    diag = np.zeros(NP, np.float32)
    diag[:N] = adjacency[np.arange(N), np.arange(N)]

    w_all_np = W_heads.transpose(1, 0, 2).reshape(IN_DIM, HEADS * HID)
    whT_np = W_heads.transpose(0, 2, 1).reshape(HEADS * HID, IN_DIM)
    a2hm = np.zeros((HID, 2 * HEADS), np.float32)
    for h in range(HEADS):
        a2hm[:, 2 * h] = a_heads[h, :HID, 0]
        a2hm[:, 2 * h + 1] = a_heads[h, HID:, 0]
    a2o_np = np.stack([a_out[:NCLS, 0], a_out[NCLS:, 0]], axis=1)

    shared = {
        "xT": np.ascontiguousarray(xTp),
        "w_all": np.ascontiguousarray(w_all_np),
        "whT": np.ascontiguousarray(whT_np),
        "a2h": np.ascontiguousarray(a2hm),
        "w_out": np.ascontiguousarray(W_out),
        "woT": np.ascontiguousarray(W_out.T),
        "a2o": np.ascontiguousarray(a2o_np),
        "fc1T": np.ascontiguousarray(FC1.T),
        "fc2T": np.ascontiguousarray(FC2.T),
        "ident": np.eye(128, dtype=np.float32),
        "identb": np.eye(128, dtype=np.float32).astype(bf),
    }
    in_maps = []
    for c in range(NCORES):
        m = dict(shared)
        m["adjc"] = np.ascontiguousarray(a1p[:, c * S:(c + 1) * S])
        m["xTown"] = np.ascontiguousarray(xTp[:, c * S:(c + 1) * S])
        m["diagv"] = np.ascontiguousarray(diag[c * S:(c + 1) * S, None])
        in_maps.append(m)
    return in_maps


def get_compiled(loop_n=None):
    key = ("nc", loop_n)
    if key not in _CACHE:
        _CACHE[key] = _build_nc(loop_n)
    return _CACHE[key]


def kernel(**inputs) -> np.ndarray:
    from concourse.bass_utils import run_bass_kernel_spmd

    nc = get_compiled()
    in_maps = _prep_inputs(inputs)
    res = run_bass_kernel_spmd(nc, in_maps, list(range(NCORES)))
    outs = [res.results[c]["out_own"] for c in range(NCORES)]
    full = np.concatenate(outs, axis=0)[:N]
    return full.astype(np.float32)

